# revision 26
# baseline (speedup 1.0000x reference)
"""Trainium2 Bass kernel for the ContrastiveLearningModule loss.

Math (mirrors the reference):
  P = l2norm(relu(E @ W1.T + b1) @ W2.T + b2)  rowwise over [T,V,L,N,D]
  for each node type t, anchors idx[t][v,l,:]:
    pos  = sum_{(x,y) != (v,l)} exp(z . P[t,x,y,id]/TEMP)
    negi = sum_{s' != s}        exp(z . z_{s'}   /TEMP)
    negc = sum_{o,k}            exp(z . P[o,v,l,nid]/TEMP)
    loss = log(pos+negi+negc) - log(pos);  out = sum(loss)/1440

Sharding: 24 (t,v,l) anchor groups = 12 "prio" (t<2, S=100,K=50) and 12
"rest" (t>=2, S=20,K=10).  Each core runs 3 slots:
  slot0: an exact prio group   (768 cols: 6x100 pos | 3x50 negs | 18 pad)
  slot1: mixed — cores 0-3 get the 4 leftover prio groups, cores 4-7 get
         a rest group padded to prio shape; this is the ONLY masked slot.
  slot2: an exact rest group   (256 cols: 6x20 | 3x10 | 106 pad)
Only gathered rows are projected (4.4k of 96k), pre-transposed on host.

Device-side structure per slot (w cols, S anchors, NK negs):
  psh = W1.x (PE, weight-block-major for minimal Ldweights)
  h   = relu(psh+b1)            ACT jt0 / DVE tensor_scalar jt1
  psz = W2.h (PE)
  sq  = Square(psz+b2)          ACT (both chunks) -> SBUF bf16
  n2T = per-column |z|^2, TRANSPOSED [128, w/128] via per-block ones
        matmuls (partition-parallel ln/exp instead of [1,w] row ops)
  rn  = exp(-0.5 ln(n2T)) -> PE transpose -> exp -> [w/128,128] rows
  psb = broadcast of rn across partitions (rank-1 matmuls)
  ph  = (psz + b2) * psb        DVE scalar_tensor_tensor -> bf16
  sims: pin=Z'Z [S,S], pc=Z'N [S,NK] (PE);  pos via pr=P.z elementwise
        (DVE) + per-(xy) column-sum matmuls -> pprT [S,5] (transposed!)
  exps: ACT exp(scale=2) of pin/pc/pprT into one e_all tile [S,256bf16]
        with col 255 preset to -e^2 (removes the diagonal i==j term
        exactly instead of a mask); accum_out of pprT-exp gives pos.
  den = row-reduce(e_all) (DVE); lnb = Ln([den|pos]) one ACT op;
  loss col = lnb[:,0]-lnb[:,1] (DVE) -> summed via ones-matmul at end.
Each core returns a partial loss sum; the host combines.
"""

import sys

import numpy as np

sys.path.insert(0, "/opt/trn_rl_repo")

import concourse.bacc as bacc
import concourse.bass as bass
import concourse.mybir as mybir
import concourse.tile as tile
from concourse.bass_utils import run_bass_kernel_spmd  # noqa: F401  (kept for harness)
from concourse.hw_specs import get_activation_tables as _real_gat
from concourse.masks import make_identity

_ONE_TABLE = "natural_log_exp_and_others"  # holds relu/identity/exp/ln/copy/square


def _gat_one_table(arch):
    """Restrict the act-table-load pass to a single function set that covers
    every ACT func this kernel uses, so exactly one LoadActFuncSet is
    emitted."""
    tabs = _real_gat(arch)
    if _ONE_TABLE in tabs:
        return {k: (v if k == _ONE_TABLE else set()) for k, v in tabs.items()}
    return tabs


bacc.get_activation_tables = _gat_one_table

F32 = mybir.dt.float32
BF16 = mybir.dt.bfloat16
AF = mybir.ActivationFunctionType
ALU = mybir.AluOpType
NP_BF16 = mybir.dt.np(BF16)

# Problem constants (hardcoded per harness contract).
T, V, L, N, D = 4, 2, 3, 4000, 256
TEMP = 0.5
XY = V * L  # 6

SP, KP = 100, 50      # prio group: anchors / cross-negs per other type
SR, KR = 20, 10       # rest group
NKP, NKR = 3 * KP, 3 * KR
GCP, GCR = XY * SP, XY * SR
WPR, WRE = 768, 256   # padded column counts per slot kind
NCOL = WPR + WPR + WRE  # 1792 packed columns per core
NCORES = 8
COUNT = 1440.0
NEG_BIG = -15000.0    # additive mask (slot1 only); exp(2*(x+NEG_BIG)) == 0
E2 = 7.3890560989306495  # e^2 = exp(sim(z,z)/TEMP) diagonal term
MMW = 512             # max matmul moving free dim into one PSUM bank

_CACHE = {}


def _stage_a(nc, consts, pools, s, kind, tag, half):
    """Projection + normalization for one half-width pass of a slot.
    half: (h_index, n_halves).  Returns ph tiles (full-slot, written per
    half via region slices)."""
    (w1, w2, bb, ones_col, ones_row, ones100f, ident, eps, nege2, mi, mcm,
     ms, xt_d) = consts
    (ppool, xpool, wpool, spool, psmm, psnb, pssim, lm, _scr) = pools

    w = WPR if kind == "P" else WRE
    hi, nh = half
    hw = w // nh
    hsl = slice(hi * hw, (hi + 1) * hw)
    o = s * 2 * WPR  # column offset of this slot in xt (slots 0,1 are P)

    # ---- projection (this half's columns of both D-chunks) ----
    xtile = xpool.tile([128, 2 * WPR], BF16, name=f"xt{tag}", tag="xtile") \
        if hi == 0 else pools[-1]["xtile"]
    if hi == 0:
        pools[-1]["xtile"] = xtile
    nc.scalar.dma_start(xtile[:, hi * hw:(hi + 1) * hw],
                        xt_d[:, o + hi * hw:o + (hi + 1) * hw])
    nc.sync.dma_start(xtile[:, w + hi * hw:w + (hi + 1) * hw],
                      xt_d[:, o + w + hi * hw:o + w + (hi + 1) * hw])
    xs = [xtile[:, hi * hw:(hi + 1) * hw],
          xtile[:, w + hi * hw:w + (hi + 1) * hw]]

    psmm0, psmm1 = psmm
    psh = [p.tile([128, 384], F32, name=f"ph{jt}{tag}h{hi}", tag=f"pph{jt}")[:, 0:hw]
           for jt, p in ((0, psmm0[0]), (1, psmm1[0]))]
    for c in (0, 1):
        for jt in (0, 1):
            nc.tensor.matmul(psh[jt], w1[c][:, jt * 128:(jt + 1) * 128],
                             xs[c], start=(c == 0), stop=(c == 1))
    if hi == 0:
        hs = [wpool.tile([128, WPR], BF16, name=f"h{jt}{tag}", tag=f"h{jt}")
              for jt in (0, 1)]
        pools[-1]["hs"] = hs
    else:
        hs = pools[-1]["hs"]
    nc.scalar.activation(hs[0][:, hsl], psh[0], AF.Relu, bias=bb[:, 0:1])
    nc.vector.tensor_scalar(hs[1][:, hsl], psh[1], bb[:, 1:2], 0.0,
                            op0=ALU.add, op1=ALU.max)

    psz = [p.tile([128, 384], F32, name=f"pz{jt}{tag}h{hi}", tag=f"pz{jt}")[:, 0:hw]
           for jt, p in ((0, psmm0[1]), (1, psmm1[1]))]
    for c in (0, 1):
        for jt in (0, 1):
            nc.tensor.matmul(psz[jt], w2[c][:, jt * 128:(jt + 1) * 128],
                             hs[c][:, hsl], start=(c == 0), stop=(c == 1))

    # ---- z to SBUF (unnormalized), squares, norms ----
    if hi == 0:
        nrm = psnb.tile([128, 512], F32, name=f"nrm{tag}", tag="nrm")
        pools[-1]["nrm"] = nrm
        zs = [wpool.tile([128, WPR], BF16, name=f"z{jt}{tag}", tag=f"z{jt}")
              for jt in (0, 1)]
        pools[-1]["zs"] = zs
        sqs = [wpool.tile([128, WPR], BF16, name=f"sq{jt}{tag}", tag=f"sq{jt}")
               for jt in (0, 1)]
        pools[-1]["sqs"] = sqs
    else:
        nrm = pools[-1]["nrm"]
        zs = pools[-1]["zs"]
        sqs = pools[-1]["sqs"]
    nc.scalar.activation(zs[0][:, hsl], psz[0], AF.Identity, bias=bb[:, 2:3])
    nc.vector.tensor_scalar_add(zs[1][:, hsl], psz[1], bb[:, 3:4])
    nc.vector.tensor_mul(sqs[0][:, hsl], zs[0][:, hsl], zs[0][:, hsl])
    nc.gpsimd.tensor_mul(sqs[1][:, hsl], zs[1][:, hsl], zs[1][:, hsl])
    nblk0 = hi * (hw // 128)
    nblkh = hw // 128
    n2T = nrm[:, 0:8]
    psrn = nrm[0:1, 8:8 + 384]  # f32 row region, reused by both halves
    for b in range(nblk0, nblk0 + nblkh):
        blk = slice(b * 128, (b + 1) * 128)
        nc.tensor.matmul(n2T[:, b:b + 1], sqs[0][:, blk], ones_col[:],
                         start=True, stop=False)
        nc.tensor.matmul(n2T[:, b:b + 1], sqs[1][:, blk], ones_col[:],
                         start=False, stop=True)
    lnT = wpool.tile([128, 8], F32, name=f"lnT{tag}h{hi}", tag="lnT")
    nc.scalar.activation(lnT[:, 0:nblkh], n2T[:, nblk0:nblk0 + nblkh],
                         AF.Ln, bias=eps[:])
    rnT = wpool.tile([128, 8], F32, name=f"rnT{tag}h{hi}", tag="rnT")
    nc.scalar.activation(rnT[:, 0:nblkh], lnT[:, 0:nblkh], AF.Exp, scale=-0.5)
    for bi in range(nblkh):
        nc.tensor.matmul(psrn[:, bi * 128:(bi + 1) * 128], rnT[:, bi:bi + 1],
                         ident[:], is_transpose=True, start=True, stop=True)
    if hi == 0:
        rnrow = wpool.tile([1, WPR], BF16, name=f"rnw{tag}", tag="rnrow")
        pools[-1]["rnrow"] = rnrow
        rnb = wpool.tile([128, WPR], BF16, name=f"rnb{tag}", tag="rnb")
        pools[-1]["rnb"] = rnb
        ph = [ppool.tile([128, WPR], BF16, name=f"phh{jt}{tag}", tag=f"phat{jt}")
              for jt in (0, 1)]
        pools[-1]["ph"] = ph
    else:
        rnrow = pools[-1]["rnrow"]
        rnb = pools[-1]["rnb"]
        ph = pools[-1]["ph"]
    if hi == 0:
        nc.vector.tensor_copy(rnrow[:, hsl], psrn[:, 0:hw])
    else:
        nc.scalar.copy(rnrow[:, hsl], psrn[:, 0:hw])
    nc.gpsimd.partition_broadcast(rnb[:, hsl], rnrow[:, hsl])

    # ---- normalize: ph = z * rnb  (all-SBUF bf16; DVE jt0, Pool jt1) ----
    nc.vector.tensor_mul(ph[0][:, hsl], zs[0][:, hsl], rnb[:, hsl])
    nc.vector.tensor_mul(ph[1][:, hsl], zs[1][:, hsl], rnb[:, hsl])
    return ph


def _stage_b(nc, consts, pools, ph, kind, masked, tag):
    """Similarities + per-anchor loss for one slot."""
    (w1, w2, bb, ones_col, ones_row, ones100f, ident, eps, nege2, mi, mcm,
     ms, xt_d) = consts
    (ppool, xpool, wpool, spool, psmm, psnb, pssim, lm, _scr) = pools
    if kind == "P":
        w, S, NK, GC = WPR, SP, NKP, GCP
    else:
        w, S, NK, GC = WRE, SR, NKR, GCR
    EW = S + NK + XY - 1  # sims + cross + pos columns, contiguous

    simt = pssim.tile([128, 512], F32, name=f"simt{tag}", tag="simt")
    pin = simt[0:S, 0:S]
    pc = simt[0:S, S:S + NK]
    for jt in (0, 1):
        nc.tensor.matmul(pin, ph[jt][:, 0:S], ph[jt][:, 0:S],
                         start=(jt == 0), stop=(jt == 1))
    for jt in (0, 1):
        nc.tensor.matmul(pc, ph[jt][:, 0:S], ph[jt][:, GC:GC + NK],
                         start=(jt == 0), stop=(jt == 1))

    # positives: pr = P_pos * z (broadcast over xy), column sums per xy via
    # rank-1 matmuls into the TRANSPOSED pprT [S, 5] right after pc
    prs = []
    for jt, eng in ((0, nc.vector), (1, nc.gpsimd)):
        pr = wpool.tile([128, (XY - 1) * SP], BF16, name=f"pr{jt}{tag}",
                        tag=f"pr{jt}")[:, 0:(XY - 1) * S]
        zb = ph[jt][:, 0:S].unsqueeze(1).to_broadcast([128, XY - 1, S])
        eng.tensor_mul(
            pr.rearrange("p (a b) -> p a b", a=XY - 1),
            ph[jt][:, S:XY * S].rearrange("p (a b) -> p a b", a=XY - 1), zb)
        prs.append(pr)

    if masked:
        # masked sims written next to the raw ones; pprT lands after them so
        # one contiguous exp covers [mask-region | pos]
        base = 256
        msk = simt[0:S, base:base + S + NK]
        nc.vector.tensor_add(msk, simt[0:S, 0:S + NK], mi[:, 0:S + NK])
        pprT = simt[0:S, base + S + NK:base + EW]
    else:
        base = 0
        pprT = simt[0:S, S + NK:EW]
    for xy in range(XY - 1):
        for jt in (0, 1):
            nc.tensor.matmul(pprT[:, xy:xy + 1],
                             prs[jt][:, xy * S:(xy + 1) * S], ones_col[:],
                             start=(jt == 0), stop=(jt == 1))

    # one exp over [sims|cross|pos]: accum_out = pos + neg_in + neg_c
    # (includes the diagonal e^2, removed by the Ln bias below); a second
    # small exp of just the pos block gives pos.
    esc = spool.tile([SP, SP + NKP + 2 * XY], BF16, name=f"esc{tag}", tag="esc")
    denb = spool.tile([SP, 2], F32, name=f"dn{tag}", tag="dn")
    nc.scalar.activation(esc[0:S, 0:EW], simt[0:S, base:base + EW], AF.Exp,
                         scale=2.0, accum_out=denb[0:S, 0:1])
    nc.scalar.activation(esc[0:S, EW:EW + XY - 1], pprT, AF.Exp,
                         scale=2.0, accum_out=denb[0:S, 1:2])

    lnb = spool.tile([SP, 2], F32, name=f"lnb{tag}", tag="lnb")
    dbias = 0.0 if masked else nege2[0:S, :]
    nc.scalar.activation(lnb[0:S, 0:1], denb[0:S, 0:1], AF.Ln, bias=dbias)
    nc.scalar.activation(lnb[0:S, 1:2], denb[0:S, 1:2], AF.Ln)
    if masked:
        lv = spool.tile([SP, 1], F32, name=f"lv{tag}", tag="lv")
        nc.vector.tensor_sub(lv[:], lnb[:, 0:1], lnb[:, 1:2])
        nc.vector.tensor_mul(lm[0][:, 1:2], lv[:], ms[:])
    elif kind == "P":
        nc.vector.tensor_sub(lm[0][:, 0:1], lnb[:, 0:1], lnb[:, 1:2])
    else:
        nc.vector.tensor_sub(lm[1][:], lnb[0:S, 0:1], lnb[0:S, 1:2])
    return simt


def _emit_body(nc, tc, consts, dram, pools, rep):
    (w1, w2, bb, ones_col, ones_row, ones100f, ident, eps, nege2, mk) = consts
    xt_d, mk_d, out_d = dram
    tag0 = f"r{rep}"
    (ppool, xpool, wpool, spool, psmm, psnb, pssim) = pools

    mi = mk[:, 0:SP + NKP]
    mcm = None
    ms = mk[:, SP + NKP:SP + NKP + 1]

    lmP = spool.tile([SP, 2], F32, name=f"lmP{tag0}", tag="lmP")
    lmR = spool.tile([SR, 1], F32, name=f"lmR{tag0}", tag="lmR")
    lm = (lmP, lmR)

    sc = (w1, w2, bb, ones_col, ones_row, ones100f, ident, eps, nege2,
          mi, mcm, ms, xt_d)

    def pl():
        return pools + (lm, {})

    pa, pb, pc_ = pl(), pl(), pl()
    _stage_a(nc, sc, pa, 0, "P", f"a{tag0}", (0, 2))
    if rep == 0:
        nc.scalar.dma_start(mk.tensor.ap()[0:SP, :], mk_d[:])
    _stage_a(nc, sc, pa, 0, "P", f"a{tag0}", (1, 2))
    _stage_a(nc, sc, pb, 1, "P", f"b{tag0}", (0, 2))
    ph0 = pa[-1]["ph"]
    _stage_b(nc, sc, pa, ph0, "P", False, f"a{tag0}")
    _stage_a(nc, sc, pb, 1, "P", f"b{tag0}", (1, 2))
    _stage_a(nc, sc, pc_, 2, "R", f"c{tag0}", (0, 1))
    ph1 = pb[-1]["ph"]
    _stage_b(nc, sc, pb, ph1, "P", True, f"b{tag0}")
    ph2 = pc_[-1]["ph"]
    simt2 = _stage_b(nc, sc, pc_, ph2, "R", False, f"c{tag0}")

    tot = simt2[0:1, 508:512]
    nc.tensor.matmul(tot[:, 0:2], ones100f[:], lmP[:], start=True, stop=True)
    nc.tensor.matmul(tot[:, 2:3], ones100f[0:SR, :], lmR[:], start=True, stop=True)
    osb = spool.tile([1, 1], F32, name=f"osb{tag0}", tag="osb")
    nc.vector.reduce_sum(osb[:], tot[:, 0:3], axis=mybir.AxisListType.X)
    nc.sync.dma_start(out_d[:], osb[:])


def _build_nc(reps=1, loop_iters=None):
    nc = bacc.Bacc("TRN2", target_bir_lowering=False, debug=False)

    xt_d = nc.dram_tensor("xt", [128, 2 * NCOL], BF16, kind="ExternalInput")
    wp_d = nc.dram_tensor("wp", [128, 4 * D], BF16, kind="ExternalInput")
    bb_d = nc.dram_tensor("bb", [128, 4], F32, kind="ExternalInput")
    mk_d = nc.dram_tensor("mk", [SP, SP + NKP + 1], F32, kind="ExternalInput")
    out_d = nc.dram_tensor("out", [1, 1], F32, kind="ExternalOutput")

    with tile.TileContext(nc) as tc:
        with tc.tile_pool(name="const", bufs=1) as cpool:
            wtile = cpool.tile([128, 4 * D], BF16, name="wtile", tag="wtile")
            nc.sync.dma_start(wtile[:], wp_d[:])
            w1 = [wtile[:, 0:D], wtile[:, D:2 * D]]
            w2 = [wtile[:, 2 * D:3 * D], wtile[:, 3 * D:4 * D]]
            bb = cpool.tile([128, 4], F32, name="bb", tag="bb")
            nc.scalar.dma_start(bb[:], bb_d[:])
            ones_col = cpool.tile([128, 1], BF16, name="ones_col", tag="ones_col")
            nc.vector.memset(ones_col[:], 1.0)
            ones_row = cpool.tile([1, 128], BF16, name="ones_row", tag="ones_row")
            nc.vector.memset(ones_row[:], 1.0)
            ones100f = cpool.tile([SP, 1], F32, name="ones100f", tag="ones100f")
            nc.vector.memset(ones100f[:], 1.0)
            ident = cpool.tile([128, 128], F32, name="ident", tag="ident")
            make_identity(nc, ident[:])
            eps = cpool.tile([128, 1], F32, name="eps", tag="eps")
            nc.vector.memset(eps[:], 1e-24)
            nege2 = cpool.tile([128, 1], F32, name="nege2", tag="nege2")
            nc.vector.memset(nege2[:], -E2)
            mk = cpool.tile([SP, SP + NKP + 1], F32, name="mk", tag="mk")

            consts = (w1, w2, bb, ones_col, ones_row, ones100f, ident, eps, nege2, mk)
            dram = (xt_d, mk_d, out_d)
            with (
                tc.tile_pool(name="phat", bufs=2) as ppool,
                tc.tile_pool(name="xin", bufs=2) as xpool,
                tc.tile_pool(name="work", bufs=2) as wpool,
                tc.tile_pool(name="sbs", bufs=2) as spool,
                tc.tile_pool(name="psh0", bufs=1, space=bass.MemorySpace.PSUM) as psh0p,
                tc.tile_pool(name="psh1", bufs=1, space=bass.MemorySpace.PSUM) as psh1p,
                tc.tile_pool(name="psz0", bufs=2, space=bass.MemorySpace.PSUM) as psz0p,
                tc.tile_pool(name="psz1", bufs=2, space=bass.MemorySpace.PSUM) as psz1p,
                tc.tile_pool(name="psnb", bufs=1, space=bass.MemorySpace.PSUM) as psnb,
                tc.tile_pool(name="pss", bufs=1, space=bass.MemorySpace.PSUM) as pssim,
            ):
                pools = (ppool, xpool, wpool, spool, ((psh0p, psz0p), (psh1p, psz1p)), psnb, pssim)
                if loop_iters is not None:
                    with tc.For_i(0, loop_iters, 1,
                                  hint_engines=(mybir.EngineType.PE,
                                                mybir.EngineType.DVE,
                                                mybir.EngineType.Activation)):
                        _emit_body(nc, tc, consts, dram, pools, 0)
                else:
                    for rep in range(reps):
                        _emit_body(nc, tc, consts, dram, pools, rep)

    nc.compile()
    return nc


def _get_nc(reps=1, loop_iters=None):
    key = ("nc", reps, loop_iters)
    if key not in _CACHE:
        _CACHE[key] = _build_nc(reps, loop_iters)
    return _CACHE[key]


def _groups():
    """Per-core slot assignment: [(group, kind, masked), ...] x3."""
    pri = [(t, v, l) for t in (0, 1) for v in range(V) for l in range(L)]
    rst = [(t, v, l) for t in (2, 3) for v in range(V) for l in range(L)]
    out = []
    for c in range(NCORES):
        s1 = pri[c]
        s2 = pri[8 + c] if c < 4 else rst[c - 4]
        s3 = rst[4 + c]
        out.append([s1, s2, s3])
    return out


def _slot_cols(E, idxp, idxr, nidxp, nidxr, g, w, Spad, Kpad):
    """Build the [w, D] gathered+padded column block for one group."""
    t, v, l = g
    if t < 2:
        idx, nid, Sr, Kr = idxp[t], nidxp[t], SP, KP
    else:
        idx, nid, Sr, Kr = idxr[t - 2], nidxr[t - 2], SR, KR
    GC = XY * Spad
    X = np.empty((w, D), np.float32)
    ids = np.asarray(idx[v, l])
    ids_p = np.concatenate([ids, np.full(Spad - Sr, ids[0], ids.dtype)])
    xy_list = [(v, l)] + [(x, y) for x in range(V) for y in range(L)
                          if (x, y) != (v, l)]
    for j, (x, y) in enumerate(xy_list):
        X[j * Spad:(j + 1) * Spad] = E[t, x, y, ids_p]
    others = [u for u in range(T) if u != t]
    for oi, u in enumerate(others):
        nk = np.asarray(nid[v, l, oi])
        nk_p = np.concatenate([nk, np.full(Kpad - Kr, nk[0], nk.dtype)])
        X[GC + oi * Kpad:GC + (oi + 1) * Kpad] = E[u, v, l, nk_p]
    X[GC + 3 * Kpad:w] = X[0]  # pad columns: dup of col 0
    return X, Sr, Kr


def make_in_maps(node_embeddings, W1, b1, W2, b2, idx_prio, idx_rest,
                 neg_idx_prio, neg_idx_rest):
    E = np.asarray(node_embeddings, dtype=np.float32)
    W1 = np.asarray(W1, dtype=np.float32)
    b1 = np.asarray(b1, dtype=np.float32)
    W2 = np.asarray(W2, dtype=np.float32)
    b2 = np.asarray(b2, dtype=np.float32)
    idxp = np.asarray(idx_prio)
    idxr = np.asarray(idx_rest)
    nidxp = np.asarray(neg_idx_prio)
    nidxr = np.asarray(neg_idx_rest)

    w1t = W1.T
    w2t = W2.T
    wp = np.concatenate([w1t[:128], w1t[128:], w2t[:128], w2t[128:]],
                        axis=1).astype(NP_BF16)
    wp = np.ascontiguousarray(wp)
    bbm = np.stack([b1[:128], b1[128:], b2[:128], b2[128:]], axis=1)
    bbm = np.ascontiguousarray(bbm, dtype=np.float32)

    in_maps = []
    for gs in _groups():
        XP = np.empty((128, 2 * NCOL), NP_BF16)
        off = 0
        MK = np.zeros((SP, SP + NKP + 1), np.float32)
        for si, g in enumerate(gs):
            if si < 2:
                w, Spad, Kpad = WPR, SP, KP
            else:
                w, Spad, Kpad = WRE, SR, KR
            X, Sr, Kr = _slot_cols(E, idxp, idxr, nidxp, nidxr, g, w, Spad, Kpad)
            if si == 1:
                # masks for the mixed slot: within-type validity + diag,
                # cross-type validity, anchor validity
                MI = np.full((SP, SP), NEG_BIG, np.float32)
                MI[:, :Sr] = 0.0
                MI[np.arange(SP), np.arange(SP)] = NEG_BIG
                MC = np.full((SP, NKP), NEG_BIG, np.float32)
                for oi in range(3):
                    MC[:, oi * KP:oi * KP + Kr] = 0.0
                MS = np.zeros((SP, 1), np.float32)
                MS[:Sr] = 1.0
                MK = np.concatenate([MI, MC, MS], axis=1).astype(np.float32)
            XT = X.T.astype(NP_BF16)  # [D, w]
            XP[:, off:off + w] = XT[0:128]
            XP[:, off + w:off + 2 * w] = XT[128:256]
            off += 2 * w
        in_maps.append({
            "xt": np.ascontiguousarray(XP),
            "wp": wp, "bb": bbm,
            "mk": np.ascontiguousarray(MK),
        })
    return in_maps


def _make_runner(nc):
    """Lower nc to a cached jitted SPMD executable."""
    import jax
    from jax.experimental.shard_map import shard_map
    from jax.sharding import Mesh, PartitionSpec

    from concourse import bass2jax
    from concourse import mybir as mb

    bass2jax.install_neuronx_cc_hook()
    partition_name = (nc.partition_id_tensor.name
                      if nc.partition_id_tensor else None)
    in_names, out_names, out_avals = [], [], []
    for alloc in nc.m.functions[0].allocations:
        if not isinstance(alloc, mb.MemoryLocationSet):
            continue
        name = alloc.memorylocations[0].name
        if alloc.kind == "ExternalInput":
            if name != partition_name:
                in_names.append(name)
        elif alloc.kind == "ExternalOutput":
            out_names.append(name)
            out_avals.append(jax.core.ShapedArray(
                tuple(alloc.tensor_shape), mb.dt.np(alloc.dtype)))
    n_params = len(in_names)
    n_outs = len(out_avals)
    all_in_names = list(in_names) + list(out_names)
    if partition_name is not None:
        all_in_names.append(partition_name)

    def _body(*args):
        operands = list(args)
        if partition_name is not None:
            operands.append(bass2jax.partition_id_tensor())
        return tuple(bass2jax._bass_exec_p.bind(
            *operands,
            out_avals=tuple(out_avals),
            in_names=tuple(all_in_names),
            out_names=tuple(out_names),
            lowering_input_output_aliases=(),
            sim_require_finite=True,
            sim_require_nnan=True,
            nc=nc,
        ))

    devices = jax.devices()[:NCORES]
    mesh = Mesh(np.asarray(devices), ("core",))
    donate = tuple(range(n_params, n_params + n_outs))
    sharded = jax.jit(
        shard_map(_body, mesh=mesh,
                  in_specs=(PartitionSpec("core"),) * (n_params + n_outs),
                  out_specs=(PartitionSpec("core"),) * n_outs,
                  check_rep=False),
        donate_argnums=donate, keep_unused=True)

    def run(in_maps, device_inputs=None):
        if device_inputs is None:
            device_inputs = [
                np.concatenate([np.asarray(m[name]) for m in in_maps], axis=0)
                for name in in_names]
        zeros = [np.zeros((NCORES * a.shape[0], *a.shape[1:]), a.dtype)
                 for a in out_avals]
        out_arrs = sharded(*device_inputs, *zeros)
        return [
            {name: np.asarray(out_arrs[i]).reshape(NCORES, *out_avals[i].shape)[c]
             for i, name in enumerate(out_names)}
            for c in range(NCORES)
        ]

    run.in_names = in_names
    run.mesh = mesh
    return run


def _get_runner(reps=1, loop_iters=None):
    key = ("runner", reps, loop_iters)
    if key not in _CACHE:
        _CACHE[key] = _make_runner(_get_nc(reps, loop_iters))
    return _CACHE[key]


class _Res:
    def __init__(self, results):
        self.results = results


def run_on_hw(in_maps, reps=1, device_inputs=None, loop_iters=None):
    runner = _get_runner(reps, loop_iters)
    return _Res(runner(in_maps, device_inputs=device_inputs))


def kernel(node_embeddings, W1, b1, W2, b2, idx_prio, idx_rest,
           neg_idx_prio, neg_idx_rest, num_views=2, num_layers=3):
    in_maps = make_in_maps(node_embeddings, W1, b1, W2, b2, idx_prio, idx_rest,
                           neg_idx_prio, neg_idx_rest)
    res = run_on_hw(in_maps)
    _CACHE["last_results"] = res
    total = sum(float(res.results[c]["out"][0, 0]) for c in range(NCORES))
    return np.float32(total / COUNT)


# revision 29
# speedup vs baseline: 2.8585x; 2.8585x over previous
"""Trainium2 Bass kernel for the ContrastiveLearningModule loss.

Math (mirrors the reference):
  P = l2norm(relu(E @ W1.T + b1) @ W2.T + b2)  rowwise over [T,V,L,N,D]
  for each node type t, anchors idx[t][v,l,:]:
    pos  = sum_{(x,y) != (v,l)} exp(z . P[t,x,y,id]/TEMP)
    negi = sum_{s' != s}        exp(z . z_{s'}   /TEMP)
    negc = sum_{o,k}            exp(z . P[o,v,l,nid]/TEMP)
    loss = log(pos+negi+negc) - log(pos);  out = sum(loss)/1440

Sharding: 24 (t,v,l) anchor groups = 12 "prio" (t<2, S=100,K=50) and 12
"rest" (t>=2, S=20,K=10).  Each core runs 3 slots:
  slot0: an exact prio group   (768 cols: 6x100 pos | 3x50 negs | 18 pad)
  slot1: mixed — cores 0-3 get the 4 leftover prio groups, cores 4-7 get
         a rest group padded to prio shape; this is the ONLY masked slot.
  slot2: an exact rest group   (256 cols: 6x20 | 3x10 | 106 pad)
Only gathered rows are projected (4.4k of 96k), pre-transposed on host.

Device-side structure per slot (w cols, S anchors, NK negs):
  psh = W1.x (PE, weight-block-major for minimal Ldweights)
  h   = relu(psh+b1)            ACT jt0 / DVE tensor_scalar jt1
  psz = W2.h (PE)
  sq  = Square(psz+b2)          ACT (both chunks) -> SBUF bf16
  n2T = per-column |z|^2, TRANSPOSED [128, w/128] via per-block ones
        matmuls (partition-parallel ln/exp instead of [1,w] row ops)
  rn  = exp(-0.5 ln(n2T)) -> PE transpose -> exp -> [w/128,128] rows
  psb = broadcast of rn across partitions (rank-1 matmuls)
  ph  = (psz + b2) * psb        DVE scalar_tensor_tensor -> bf16
  sims: pin=Z'Z [S,S], pc=Z'N [S,NK] (PE);  pos via pr=P.z elementwise
        (DVE) + per-(xy) column-sum matmuls -> pprT [S,5] (transposed!)
  exps: ACT exp(scale=2) of pin/pc/pprT into one e_all tile [S,256bf16]
        with col 255 preset to -e^2 (removes the diagonal i==j term
        exactly instead of a mask); accum_out of pprT-exp gives pos.
  den = row-reduce(e_all) (DVE); lnb = Ln([den|pos]) one ACT op;
  loss col = lnb[:,0]-lnb[:,1] (DVE) -> summed via ones-matmul at end.
Each core returns a partial loss sum; the host combines.
"""

import sys

import numpy as np

sys.path.insert(0, "/opt/trn_rl_repo")

import concourse.bacc as bacc
import concourse.bass as bass
import concourse.mybir as mybir
import concourse.tile as tile
from concourse.bass_utils import run_bass_kernel_spmd  # noqa: F401  (kept for harness)
from concourse.hw_specs import get_activation_tables as _real_gat
from concourse.masks import make_identity

_ONE_TABLE = "natural_log_exp_and_others"  # holds relu/identity/exp/ln/copy/square


def _gat_one_table(arch):
    """Restrict the act-table-load pass to a single function set that covers
    every ACT func this kernel uses, so exactly one LoadActFuncSet is
    emitted."""
    tabs = _real_gat(arch)
    if _ONE_TABLE in tabs:
        return {k: (v if k == _ONE_TABLE else set()) for k, v in tabs.items()}
    return tabs


bacc.get_activation_tables = _gat_one_table

F32 = mybir.dt.float32
BF16 = mybir.dt.bfloat16
AF = mybir.ActivationFunctionType
ALU = mybir.AluOpType
NP_BF16 = mybir.dt.np(BF16)

# Problem constants (hardcoded per harness contract).
T, V, L, N, D = 4, 2, 3, 4000, 256
TEMP = 0.5
XY = V * L  # 6

SP, KP = 100, 50      # prio group: anchors / cross-negs per other type
SR, KR = 20, 10       # rest group
NKP, NKR = 3 * KP, 3 * KR
GCP, GCR = XY * SP, XY * SR
WPR, WRE = 768, 256   # padded column counts per slot kind
NCOL = WPR + WPR + WRE  # 1792 packed columns per core
NCORES = 8
COUNT = 1440.0
NEG_BIG = -15000.0    # additive mask (slot1 only); exp(2*(x+NEG_BIG)) == 0
E2 = 7.3890560989306495  # e^2 = exp(sim(z,z)/TEMP) diagonal term
MMW = 512             # max matmul moving free dim into one PSUM bank

_CACHE = {}


def _stage_a(nc, consts, pools, s, kind, tag, half):
    """Projection + normalization for one half-width pass of a slot.
    half: (h_index, n_halves).  Returns ph tiles (full-slot, written per
    half via region slices)."""
    (w1, w2, bb, ones_col, ones_row, ones100f, ident, eps, nege2, mi, mcm,
     ms, xt_d) = consts
    (ppool, xpool, wpool, spool, psmm, psnb, pssim, lm, _scr) = pools

    w = WPR if kind == "P" else WRE
    hi, nh = half
    hw = w // nh
    hsl = slice(hi * hw, (hi + 1) * hw)
    o = s * 2 * WPR  # column offset of this slot in xt (slots 0,1 are P)

    # ---- projection (this half's columns of both D-chunks) ----
    xtile = xpool.tile([128, 2 * WPR], BF16, name=f"xt{tag}", tag="xtile") \
        if hi == 0 else pools[-1]["xtile"]
    if hi == 0:
        pools[-1]["xtile"] = xtile
    nc.sync.dma_start(xtile[:, hi * hw:(hi + 1) * hw],
                      xt_d[:, o + hi * hw:o + (hi + 1) * hw])
    nc.sync.dma_start(xtile[:, w + hi * hw:w + (hi + 1) * hw],
                      xt_d[:, o + w + hi * hw:o + w + (hi + 1) * hw])
    xs = [xtile[:, hi * hw:(hi + 1) * hw],
          xtile[:, w + hi * hw:w + (hi + 1) * hw]]

    psmm0, psmm1 = psmm
    psh = [p.tile([128, 384], F32, name=f"ph{jt}{tag}h{hi}", tag=f"pph{jt}")[:, 0:hw]
           for jt, p in ((0, psmm0[0]), (1, psmm1[0]))]
    for c in (0, 1):
        for jt in (0, 1):
            nc.tensor.matmul(psh[jt], w1[c][:, jt * 128:(jt + 1) * 128],
                             xs[c], start=(c == 0), stop=(c == 1))
    if hi == 0:
        hs = [wpool.tile([128, WPR], BF16, name=f"h{jt}{tag}", tag=f"h{jt}")
              for jt in (0, 1)]
        pools[-1]["hs"] = hs
    else:
        hs = pools[-1]["hs"]
    nc.scalar.activation(hs[0][:, hsl], psh[0], AF.Relu, bias=bb[:, 0:1])
    nc.vector.tensor_scalar(hs[1][:, hsl], psh[1], bb[:, 1:2], 0.0,
                            op0=ALU.add, op1=ALU.max)

    psz = [p.tile([128, 384], F32, name=f"pz{jt}{tag}h{hi}", tag=f"pz{jt}")[:, 0:hw]
           for jt, p in ((0, psmm0[1]), (1, psmm1[1]))]
    for c in (0, 1):
        for jt in (0, 1):
            nc.tensor.matmul(psz[jt], w2[c][:, jt * 128:(jt + 1) * 128],
                             hs[c][:, hsl], start=(c == 0), stop=(c == 1))

    # ---- z to SBUF (unnormalized), squares, norms ----
    if hi == 0:
        nrm = psnb.tile([128, 512], F32, name=f"nrm{tag}", tag="nrm")
        pools[-1]["nrm"] = nrm
        zs = [wpool.tile([128, WPR], BF16, name=f"z{jt}{tag}", tag=f"z{jt}")
              for jt in (0, 1)]
        pools[-1]["zs"] = zs
        sqs = [wpool.tile([128, WPR], BF16, name=f"sq{jt}{tag}", tag=f"sq{jt}")
               for jt in (0, 1)]
        pools[-1]["sqs"] = sqs
    else:
        nrm = pools[-1]["nrm"]
        zs = pools[-1]["zs"]
        sqs = pools[-1]["sqs"]
    nc.scalar.activation(zs[0][:, hsl], psz[0], AF.Identity, bias=bb[:, 2:3])
    nc.vector.tensor_scalar_add(zs[1][:, hsl], psz[1], bb[:, 3:4])
    nc.vector.tensor_mul(sqs[0][:, hsl], zs[0][:, hsl], zs[0][:, hsl])
    nc.vector.tensor_mul(sqs[1][:, hsl], zs[1][:, hsl], zs[1][:, hsl])
    nblk0 = hi * (hw // 128)
    nblkh = hw // 128
    n2T = nrm[:, 0:8]
    psrn = nrm[0:1, 8:8 + 384]  # f32 row region, reused by both halves
    for b in range(nblk0, nblk0 + nblkh):
        blk = slice(b * 128, (b + 1) * 128)
        nc.tensor.matmul(n2T[:, b:b + 1], sqs[0][:, blk], ones_col[:],
                         start=True, stop=False)
        nc.tensor.matmul(n2T[:, b:b + 1], sqs[1][:, blk], ones_col[:],
                         start=False, stop=True)
    lnT = wpool.tile([128, 8], F32, name=f"lnT{tag}h{hi}", tag="lnT")
    nc.scalar.activation(lnT[:, 0:nblkh], n2T[:, nblk0:nblk0 + nblkh],
                         AF.Ln, bias=eps[:])
    rnT = wpool.tile([128, 8], F32, name=f"rnT{tag}h{hi}", tag="rnT")
    nc.scalar.activation(rnT[:, 0:nblkh], lnT[:, 0:nblkh], AF.Exp, scale=-0.5)
    for bi in range(nblkh):
        nc.tensor.matmul(psrn[:, bi * 128:(bi + 1) * 128], rnT[:, bi:bi + 1],
                         ident[:], is_transpose=True, start=True, stop=True)
    if hi == 0:
        rnrow = wpool.tile([1, WPR], BF16, name=f"rnw{tag}", tag="rnrow")
        pools[-1]["rnrow"] = rnrow
        rnb = wpool.tile([128, WPR], BF16, name=f"rnb{tag}", tag="rnb")
        pools[-1]["rnb"] = rnb
        ph = [ppool.tile([128, WPR], BF16, name=f"phh{jt}{tag}", tag=f"phat{jt}")
              for jt in (0, 1)]
        pools[-1]["ph"] = ph
    else:
        rnrow = pools[-1]["rnrow"]
        rnb = pools[-1]["rnb"]
        ph = pools[-1]["ph"]
    if hi == 0:
        nc.vector.tensor_copy(rnrow[:, hsl], psrn[:, 0:hw])
    else:
        nc.scalar.copy(rnrow[:, hsl], psrn[:, 0:hw])
    nc.gpsimd.partition_broadcast(rnb[:, hsl], rnrow[:, hsl])

    # ---- normalize: ph = z * rnb  (all-SBUF bf16; DVE jt0, Pool jt1) ----
    nc.vector.tensor_mul(ph[0][:, hsl], zs[0][:, hsl], rnb[:, hsl])
    nc.vector.tensor_mul(ph[1][:, hsl], zs[1][:, hsl], rnb[:, hsl])
    return ph


def _stage_b(nc, consts, pools, ph, kind, masked, tag):
    """Similarities + per-anchor loss for one slot."""
    (w1, w2, bb, ones_col, ones_row, ones100f, ident, eps, nege2, mi, mcm,
     ms, xt_d) = consts
    (ppool, xpool, wpool, spool, psmm, psnb, pssim, lm, _scr) = pools
    if kind == "P":
        w, S, NK, GC = WPR, SP, NKP, GCP
    else:
        w, S, NK, GC = WRE, SR, NKR, GCR
    EW = S + NK + XY - 1  # sims + cross + pos columns, contiguous

    simt = pssim.tile([128, 512], F32, name=f"simt{tag}", tag="simt")
    pin = simt[0:S, 0:S]
    pc = simt[0:S, S:S + NK]
    for jt in (0, 1):
        nc.tensor.matmul(pin, ph[jt][:, 0:S], ph[jt][:, 0:S],
                         start=(jt == 0), stop=(jt == 1))
    for jt in (0, 1):
        nc.tensor.matmul(pc, ph[jt][:, 0:S], ph[jt][:, GC:GC + NK],
                         start=(jt == 0), stop=(jt == 1))

    # positives: pr = P_pos * z (broadcast over xy), column sums per xy via
    # rank-1 matmuls into the TRANSPOSED pprT [S, 5] right after pc
    prs = []
    for jt, eng in ((0, nc.vector), (1, nc.vector)):
        pr = wpool.tile([128, (XY - 1) * SP], BF16, name=f"pr{jt}{tag}",
                        tag=f"pr{jt}")[:, 0:(XY - 1) * S]
        zb = ph[jt][:, 0:S].unsqueeze(1).to_broadcast([128, XY - 1, S])
        eng.tensor_mul(
            pr.rearrange("p (a b) -> p a b", a=XY - 1),
            ph[jt][:, S:XY * S].rearrange("p (a b) -> p a b", a=XY - 1), zb)
        prs.append(pr)

    if masked:
        # masked sims written next to the raw ones; pprT lands after them so
        # one contiguous exp covers [mask-region | pos]
        base = 256
        msk = simt[0:S, base:base + S + NK]
        nc.vector.tensor_add(msk, simt[0:S, 0:S + NK], mi[:, 0:S + NK])
        pprT = simt[0:S, base + S + NK:base + EW]
    else:
        base = 0
        pprT = simt[0:S, S + NK:EW]
    for xy in range(XY - 1):
        for jt in (0, 1):
            nc.tensor.matmul(pprT[:, xy:xy + 1],
                             prs[jt][:, xy * S:(xy + 1) * S], ones_col[:],
                             start=(jt == 0), stop=(jt == 1))

    # one exp over [sims|cross|pos]: accum_out = pos + neg_in + neg_c
    # (includes the diagonal e^2, removed by the Ln bias below); a second
    # small exp of just the pos block gives pos.
    esc = spool.tile([SP, SP + NKP + 2 * XY], BF16, name=f"esc{tag}", tag="esc")
    denb = spool.tile([SP, 2], F32, name=f"dn{tag}", tag="dn")
    nc.scalar.activation(esc[0:S, 0:EW], simt[0:S, base:base + EW], AF.Exp,
                         scale=2.0, accum_out=denb[0:S, 0:1])
    nc.scalar.activation(esc[0:S, EW:EW + XY - 1], pprT, AF.Exp,
                         scale=2.0, accum_out=denb[0:S, 1:2])

    lnb = spool.tile([SP, 2], F32, name=f"lnb{tag}", tag="lnb")
    dbias = 0.0 if masked else nege2[0:S, :]
    nc.scalar.activation(lnb[0:S, 0:1], denb[0:S, 0:1], AF.Ln, bias=dbias)
    nc.scalar.activation(lnb[0:S, 1:2], denb[0:S, 1:2], AF.Ln)
    if masked:
        lv = spool.tile([SP, 1], F32, name=f"lv{tag}", tag="lv")
        nc.vector.tensor_sub(lv[:], lnb[:, 0:1], lnb[:, 1:2])
        nc.vector.tensor_mul(lm[0][:, 1:2], lv[:], ms[:])
    elif kind == "P":
        nc.vector.tensor_sub(lm[0][:, 0:1], lnb[:, 0:1], lnb[:, 1:2])
    else:
        nc.vector.tensor_sub(lm[1][:], lnb[0:S, 0:1], lnb[0:S, 1:2])
    return simt


def _emit_body(nc, tc, consts, dram, pools, rep):
    (w1, w2, bb, ones_col, ones_row, ones100f, ident, eps, nege2, mk) = consts
    xt_d, mk_d, out_d = dram
    tag0 = f"r{rep}"
    (ppool, xpool, wpool, spool, psmm, psnb, pssim) = pools

    mi = mk[:, 0:SP + NKP]
    mcm = None
    ms = mk[:, SP + NKP:SP + NKP + 1]

    lmP = spool.tile([SP, 2], F32, name=f"lmP{tag0}", tag="lmP")
    lmR = spool.tile([SR, 1], F32, name=f"lmR{tag0}", tag="lmR")
    lm = (lmP, lmR)

    sc = (w1, w2, bb, ones_col, ones_row, ones100f, ident, eps, nege2,
          mi, mcm, ms, xt_d)

    def pl():
        return pools + (lm, {})

    pa, pb, pc_ = pl(), pl(), pl()
    _stage_a(nc, sc, pa, 0, "P", f"a{tag0}", (0, 2))
    if rep == 0:
        nc.sync.dma_start(mk.tensor.ap()[0:SP, :], mk_d[:])
    _stage_a(nc, sc, pa, 0, "P", f"a{tag0}", (1, 2))
    _stage_a(nc, sc, pb, 1, "P", f"b{tag0}", (0, 2))
    ph0 = pa[-1]["ph"]
    _stage_b(nc, sc, pa, ph0, "P", False, f"a{tag0}")
    _stage_a(nc, sc, pb, 1, "P", f"b{tag0}", (1, 2))
    _stage_a(nc, sc, pc_, 2, "R", f"c{tag0}", (0, 1))
    ph1 = pb[-1]["ph"]
    _stage_b(nc, sc, pb, ph1, "P", True, f"b{tag0}")
    ph2 = pc_[-1]["ph"]
    simt2 = _stage_b(nc, sc, pc_, ph2, "R", False, f"c{tag0}")

    tot = simt2[0:1, 508:512]
    nc.tensor.matmul(tot[:, 0:2], ones100f[:], lmP[:], start=True, stop=True)
    nc.tensor.matmul(tot[:, 2:3], ones100f[0:SR, :], lmR[:], start=True, stop=True)
    osb = spool.tile([1, 1], F32, name=f"osb{tag0}", tag="osb")
    nc.vector.reduce_sum(osb[:], tot[:, 0:3], axis=mybir.AxisListType.X)
    nc.sync.dma_start(out_d[:], osb[:])


def _build_nc(reps=1, loop_iters=None):
    nc = bacc.Bacc("TRN2", target_bir_lowering=False, debug=False)

    xt_d = nc.dram_tensor("xt", [128, 2 * NCOL], BF16, kind="ExternalInput")
    wp_d = nc.dram_tensor("wp", [128, 4 * D], BF16, kind="ExternalInput")
    bb_d = nc.dram_tensor("bb", [128, 4], F32, kind="ExternalInput")
    mk_d = nc.dram_tensor("mk", [SP, SP + NKP + 1], F32, kind="ExternalInput")
    out_d = nc.dram_tensor("out", [1, 1], F32, kind="ExternalOutput")

    with tile.TileContext(nc) as tc:
        with tc.tile_pool(name="const", bufs=1) as cpool:
            wtile = cpool.tile([128, 4 * D], BF16, name="wtile", tag="wtile")
            nc.sync.dma_start(wtile[:], wp_d[:])
            w1 = [wtile[:, 0:D], wtile[:, D:2 * D]]
            w2 = [wtile[:, 2 * D:3 * D], wtile[:, 3 * D:4 * D]]
            bb = cpool.tile([128, 4], F32, name="bb", tag="bb")
            nc.scalar.dma_start(bb[:], bb_d[:])
            ones_col = cpool.tile([128, 1], BF16, name="ones_col", tag="ones_col")
            nc.vector.memset(ones_col[:], 1.0)
            ones_row = cpool.tile([1, 128], BF16, name="ones_row", tag="ones_row")
            nc.vector.memset(ones_row[:], 1.0)
            ones100f = cpool.tile([SP, 1], F32, name="ones100f", tag="ones100f")
            nc.vector.memset(ones100f[:], 1.0)
            ident = cpool.tile([128, 128], F32, name="ident", tag="ident")
            make_identity(nc, ident[:])
            eps = cpool.tile([128, 1], F32, name="eps", tag="eps")
            nc.vector.memset(eps[:], 1e-24)
            nege2 = cpool.tile([128, 1], F32, name="nege2", tag="nege2")
            nc.vector.memset(nege2[:], -E2)
            mk = cpool.tile([SP, SP + NKP + 1], F32, name="mk", tag="mk")
            nc.sync.dma_start(mk.tensor.ap()[0:SP, :], mk_d[:])

            consts = (w1, w2, bb, ones_col, ones_row, ones100f, ident, eps, nege2, mk)
            dram = (xt_d, mk_d, out_d)
            with (
                tc.tile_pool(name="phat", bufs=2) as ppool,
                tc.tile_pool(name="xin", bufs=2) as xpool,
                tc.tile_pool(name="work", bufs=2) as wpool,
                tc.tile_pool(name="sbs", bufs=2) as spool,
                tc.tile_pool(name="psh0", bufs=1, space=bass.MemorySpace.PSUM) as psh0p,
                tc.tile_pool(name="psh1", bufs=1, space=bass.MemorySpace.PSUM) as psh1p,
                tc.tile_pool(name="psz0", bufs=2, space=bass.MemorySpace.PSUM) as psz0p,
                tc.tile_pool(name="psz1", bufs=2, space=bass.MemorySpace.PSUM) as psz1p,
                tc.tile_pool(name="psnb", bufs=1, space=bass.MemorySpace.PSUM) as psnb,
                tc.tile_pool(name="pss", bufs=1, space=bass.MemorySpace.PSUM) as pssim,
            ):
                pools = (ppool, xpool, wpool, spool, ((psh0p, psz0p), (psh1p, psz1p)), psnb, pssim)
                if loop_iters is not None:
                    with tc.For_i(0, loop_iters, 1,
                                  hint_engines=(mybir.EngineType.PE,
                                                mybir.EngineType.DVE,
                                                mybir.EngineType.Pool,
                                                mybir.EngineType.Activation)):
                        _emit_body(nc, tc, consts, dram, pools, 0)
                else:
                    for rep in range(reps):
                        _emit_body(nc, tc, consts, dram, pools, rep)

    nc.compile()
    return nc


def _get_nc(reps=1, loop_iters=None):
    key = ("nc", reps, loop_iters)
    if key not in _CACHE:
        _CACHE[key] = _build_nc(reps, loop_iters)
    return _CACHE[key]


def _groups():
    """Per-core slot assignment: [(group, kind, masked), ...] x3."""
    pri = [(t, v, l) for t in (0, 1) for v in range(V) for l in range(L)]
    rst = [(t, v, l) for t in (2, 3) for v in range(V) for l in range(L)]
    out = []
    for c in range(NCORES):
        s1 = pri[c]
        s2 = pri[8 + c] if c < 4 else rst[c - 4]
        s3 = rst[4 + c]
        out.append([s1, s2, s3])
    return out


def _slot_cols(E, idxp, idxr, nidxp, nidxr, g, w, Spad, Kpad):
    """Build the [w, D] gathered+padded column block for one group."""
    t, v, l = g
    if t < 2:
        idx, nid, Sr, Kr = idxp[t], nidxp[t], SP, KP
    else:
        idx, nid, Sr, Kr = idxr[t - 2], nidxr[t - 2], SR, KR
    GC = XY * Spad
    X = np.empty((w, D), np.float32)
    ids = np.asarray(idx[v, l])
    ids_p = np.concatenate([ids, np.full(Spad - Sr, ids[0], ids.dtype)])
    xy_list = [(v, l)] + [(x, y) for x in range(V) for y in range(L)
                          if (x, y) != (v, l)]
    for j, (x, y) in enumerate(xy_list):
        X[j * Spad:(j + 1) * Spad] = E[t, x, y, ids_p]
    others = [u for u in range(T) if u != t]
    for oi, u in enumerate(others):
        nk = np.asarray(nid[v, l, oi])
        nk_p = np.concatenate([nk, np.full(Kpad - Kr, nk[0], nk.dtype)])
        X[GC + oi * Kpad:GC + (oi + 1) * Kpad] = E[u, v, l, nk_p]
    X[GC + 3 * Kpad:w] = X[0]  # pad columns: dup of col 0
    return X, Sr, Kr


def make_in_maps(node_embeddings, W1, b1, W2, b2, idx_prio, idx_rest,
                 neg_idx_prio, neg_idx_rest):
    E = np.asarray(node_embeddings, dtype=np.float32)
    W1 = np.asarray(W1, dtype=np.float32)
    b1 = np.asarray(b1, dtype=np.float32)
    W2 = np.asarray(W2, dtype=np.float32)
    b2 = np.asarray(b2, dtype=np.float32)
    idxp = np.asarray(idx_prio)
    idxr = np.asarray(idx_rest)
    nidxp = np.asarray(neg_idx_prio)
    nidxr = np.asarray(neg_idx_rest)

    w1t = W1.T
    w2t = W2.T
    wp = np.concatenate([w1t[:128], w1t[128:], w2t[:128], w2t[128:]],
                        axis=1).astype(NP_BF16)
    wp = np.ascontiguousarray(wp)
    bbm = np.stack([b1[:128], b1[128:], b2[:128], b2[128:]], axis=1)
    bbm = np.ascontiguousarray(bbm, dtype=np.float32)

    in_maps = []
    for gs in _groups():
        XP = np.empty((128, 2 * NCOL), NP_BF16)
        off = 0
        MK = np.zeros((SP, SP + NKP + 1), np.float32)
        for si, g in enumerate(gs):
            if si < 2:
                w, Spad, Kpad = WPR, SP, KP
            else:
                w, Spad, Kpad = WRE, SR, KR
            X, Sr, Kr = _slot_cols(E, idxp, idxr, nidxp, nidxr, g, w, Spad, Kpad)
            if si == 1:
                # masks for the mixed slot: within-type validity + diag,
                # cross-type validity, anchor validity
                MI = np.full((SP, SP), NEG_BIG, np.float32)
                MI[:, :Sr] = 0.0
                MI[np.arange(SP), np.arange(SP)] = NEG_BIG
                MC = np.full((SP, NKP), NEG_BIG, np.float32)
                for oi in range(3):
                    MC[:, oi * KP:oi * KP + Kr] = 0.0
                MS = np.zeros((SP, 1), np.float32)
                MS[:Sr] = 1.0
                MK = np.concatenate([MI, MC, MS], axis=1).astype(np.float32)
            XT = X.T.astype(NP_BF16)  # [D, w]
            XP[:, off:off + w] = XT[0:128]
            XP[:, off + w:off + 2 * w] = XT[128:256]
            off += 2 * w
        in_maps.append({
            "xt": np.ascontiguousarray(XP),
            "wp": wp, "bb": bbm,
            "mk": np.ascontiguousarray(MK),
        })
    return in_maps


def _make_runner(nc):
    """Lower nc to a cached jitted SPMD executable."""
    import jax
    from jax.experimental.shard_map import shard_map
    from jax.sharding import Mesh, PartitionSpec

    from concourse import bass2jax
    from concourse import mybir as mb

    bass2jax.install_neuronx_cc_hook()
    partition_name = (nc.partition_id_tensor.name
                      if nc.partition_id_tensor else None)
    in_names, out_names, out_avals = [], [], []
    for alloc in nc.m.functions[0].allocations:
        if not isinstance(alloc, mb.MemoryLocationSet):
            continue
        name = alloc.memorylocations[0].name
        if alloc.kind == "ExternalInput":
            if name != partition_name:
                in_names.append(name)
        elif alloc.kind == "ExternalOutput":
            out_names.append(name)
            out_avals.append(jax.core.ShapedArray(
                tuple(alloc.tensor_shape), mb.dt.np(alloc.dtype)))
    n_params = len(in_names)
    n_outs = len(out_avals)
    all_in_names = list(in_names) + list(out_names)
    if partition_name is not None:
        all_in_names.append(partition_name)

    def _body(*args):
        operands = list(args)
        if partition_name is not None:
            operands.append(bass2jax.partition_id_tensor())
        return tuple(bass2jax._bass_exec_p.bind(
            *operands,
            out_avals=tuple(out_avals),
            in_names=tuple(all_in_names),
            out_names=tuple(out_names),
            lowering_input_output_aliases=(),
            sim_require_finite=True,
            sim_require_nnan=True,
            nc=nc,
        ))

    devices = jax.devices()[:NCORES]
    mesh = Mesh(np.asarray(devices), ("core",))
    donate = tuple(range(n_params, n_params + n_outs))
    sharded = jax.jit(
        shard_map(_body, mesh=mesh,
                  in_specs=(PartitionSpec("core"),) * (n_params + n_outs),
                  out_specs=(PartitionSpec("core"),) * n_outs,
                  check_rep=False),
        donate_argnums=donate, keep_unused=True)

    def run(in_maps, device_inputs=None):
        if device_inputs is None:
            device_inputs = [
                np.concatenate([np.asarray(m[name]) for m in in_maps], axis=0)
                for name in in_names]
        zeros = [np.zeros((NCORES * a.shape[0], *a.shape[1:]), a.dtype)
                 for a in out_avals]
        out_arrs = sharded(*device_inputs, *zeros)
        return [
            {name: np.asarray(out_arrs[i]).reshape(NCORES, *out_avals[i].shape)[c]
             for i, name in enumerate(out_names)}
            for c in range(NCORES)
        ]

    run.in_names = in_names
    run.mesh = mesh
    return run


def _get_runner(reps=1, loop_iters=None):
    key = ("runner", reps, loop_iters)
    if key not in _CACHE:
        _CACHE[key] = _make_runner(_get_nc(reps, loop_iters))
    return _CACHE[key]


class _Res:
    def __init__(self, results):
        self.results = results


def run_on_hw(in_maps, reps=1, device_inputs=None, loop_iters=None):
    runner = _get_runner(reps, loop_iters)
    return _Res(runner(in_maps, device_inputs=device_inputs))


def kernel(node_embeddings, W1, b1, W2, b2, idx_prio, idx_rest,
           neg_idx_prio, neg_idx_rest, num_views=2, num_layers=3):
    in_maps = make_in_maps(node_embeddings, W1, b1, W2, b2, idx_prio, idx_rest,
                           neg_idx_prio, neg_idx_rest)
    res = run_on_hw(in_maps)
    _CACHE["last_results"] = res
    total = sum(float(res.results[c]["out"][0, 0]) for c in range(NCORES))
    return np.float32(total / COUNT)
